# revision 2
# baseline (speedup 1.0000x reference)
"""GAT layer kernel for Trainium2, data-parallel over batch across 8 NeuronCores.

Reference computation (per batch b):
    Wh   = x @ W                                  [N, F]
    s_src = Wh @ a_w[:F];  s_dst = Wh @ a_w[F:]   [N]
    e    = s_src[:, None] + s_dst[None, :] + a_b  [N, N]
    exps = exp(leaky_relu(e, 0.2)) * A
    attn = exps / (exps.sum(axis=0) + 1e-7)       # softmax over dim i
    out  = attn @ Wh

Device strategy (per core = one batch):
  * Host folds the mask additively into the score matrix:
        Am2[i, j] = (A[i, j] - 1) * C2 + s_src[i]          (C2 = 150)
    so masked entries get an exponent shift of -150; after leaky_relu the
    masked exponent is ~-30, exp gives ~1e-13 which flushes to 0 in fp16.
  * PE transposes Am2 blocks straight into PSUM -> z[j, i] tiles; the
    per-partition (j) bias s_dst[j] + a_b is applied by ACT/DVE.
  * leaky_relu+exp is split between ACT (Prelu -> Exp) and DVE
    ((z+bias), max(0.2w, w) -> ACT Exp) to balance engine load.
  * Exp writes expsT in fp16 with accum_out giving column sums; Wh rows are
    scaled by 1/(sums+eps) (fold the softmax division into the rhs).
  * Final matmul: out[i, :] += expsT[j, i].T @ (Wh[j, :] / sums[j]) in fp16.
"""

import numpy as np

import concourse.bass as bass
import concourse.mybir as mybir
import concourse.tile as tile
from concourse import bacc
from concourse.bass_utils import run_bass_kernel_spmd
from concourse.masks import make_identity

B, N, F = 8, 2048, 256
NT = N // 128          # 16 row/col tiles
NJG = 4                # j-tile groups (softmax sums complete per group)
JPG = NT // NJG        # j-tiles per group
C2 = 150.0
EPS = 1e-7
NEG_SLOPE = 0.2

f32 = mybir.dt.float32
f16 = mybir.dt.float16

AF = mybir.ActivationFunctionType
ALU = mybir.AluOpType


def build(nc, loop_n=None):
    am_d = nc.declare_dram_parameter("am", [N, N], f32, isOutput=False)
    xt_d = nc.declare_dram_parameter("xt", [F, N], f16, isOutput=False)
    w_d = nc.declare_dram_parameter("w16", [F, F], f16, isOutput=False)
    sdst_d = nc.declare_dram_parameter("sdstc", [128, NT], f32, isOutput=False)
    out_d = nc.declare_dram_parameter("out", [N, F], f32, isOutput=True)

    with tile.TileContext(nc) as tc:
        with (
            tc.tile_pool(name="const", bufs=1) as const,
            tc.tile_pool(name="xt", bufs=2) as xtp,
            tc.tile_pool(name="wh", bufs=NT) as whp,
            tc.tile_pool(name="whs", bufs=NT) as whsp,
            tc.tile_pool(name="expsT", bufs=NT) as expp,
            tc.tile_pool(name="amst", bufs=NT + 2) as amp,
            tc.tile_pool(name="tstream", bufs=3) as tsp,
            tc.tile_pool(name="wstream", bufs=3) as wsp,
            tc.tile_pool(name="t2stream", bufs=3) as t2p,
            tc.tile_pool(name="sums", bufs=1) as sump,
            tc.tile_pool(name="outsb", bufs=NT) as outp,
            tc.tile_pool(name="mm1ps", bufs=2, space="PSUM") as mm1ps,
            tc.tile_pool(name="trps", bufs=2, space="PSUM") as trps,
            tc.tile_pool(name="outps", bufs=2, space="PSUM") as outps,
        ):
            w16a = const.tile([128, F], f16)
            w16b = const.tile([128, F], f16)
            sdstc = const.tile([128, NT], f32)
            ident = const.tile([128, 128], f32)
            nc.sync.dma_start(w16a[:], w_d[0:128, :])
            nc.sync.dma_start(w16b[:], w_d[128:256, :])
            nc.sync.dma_start(sdstc[:], sdst_d[:])
            make_identity(nc, ident[:])

            def body(_iv=None):
                xt0 = xtp.tile([128, N], f16, tag="xt")
                xt1 = xtp.tile([128, N], f16, tag="xt")
                nc.sync.dma_start(xt0[:], xt_d[0:128, :])
                nc.sync.dma_start(xt1[:], xt_d[128:256, :])

                # ---- Wh = x @ W, tiles [128 j, 256 o] fp32 ----
                wh = []
                for nt in range(NT):
                    ps = mm1ps.tile([128, F], f32)
                    sl = slice(nt * 128, (nt + 1) * 128)
                    nc.tensor.matmul(ps[:], xt0[:, sl], w16a[:], start=True, stop=False)
                    nc.tensor.matmul(ps[:], xt1[:, sl], w16b[:], start=False, stop=True)
                    t = whp.tile([128, F], f32, tag="wh")
                    nc.vector.tensor_copy(t[:], ps[:])
                    wh.append(t)

                sums_acc = sump.tile([128, NT, 2], f32, tag="sa")
                sums_red = sump.tile([128, NT], f32, tag="sr")
                recip = sump.tile([128, NT], f32, tag="rc")
                expsT = [expp.tile([128, N], f16, tag="ex", name=f"expsT{j}") for j in range(NT)]
                outsb = [outp.tile([128, F], f32, tag="ob", name=f"outsb{i}") for i in range(NT)]

                for jg in range(NJG):
                    j0 = jg * JPG * 128
                    strips = []
                    for it in range(NT):
                        st = amp.tile([128, JPG * 128], f32, tag="am")
                        nc.sync.dma_start(
                            st[:],
                            am_d[it * 128 : (it + 1) * 128, j0 : j0 + JPG * 128],
                        )
                        strips.append(st)

                    for jl in range(JPG):
                        jt = jg * JPG + jl
                        bias = sdstc[:, jt : jt + 1]
                        for half in range(2):
                            tp = trps.tile([128, 1024], f32)
                            for k in range(8):
                                it = half * 8 + k
                                nc.tensor.transpose(
                                    tp[:, k * 128 : (k + 1) * 128],
                                    strips[it][:, jl * 128 : (jl + 1) * 128],
                                    ident[:],
                                )
                            dst = expsT[jt][:, half * 1024 : (half + 1) * 1024]
                            acc = sums_acc[:, jt : jt + 1, half : half + 1]
                            if half == 0:
                                # ACT path: t = prelu(z + bias); exp(t)
                                t = tsp.tile([128, 1024], f32, tag="ts")
                                nc.scalar.activation(
                                    t[:], tp[:], AF.Prelu,
                                    bias=bias, scale=1.0, alpha=NEG_SLOPE,
                                )
                                nc.scalar.activation(
                                    dst, t[:], AF.Exp, bias=0.0, scale=1.0,
                                    accum_out=acc,
                                )
                            else:
                                # DVE path: w = z + bias; t2 = max(.2w, w); exp
                                w = wsp.tile([128, 1024], f32, tag="ws")
                                nc.vector.tensor_scalar(
                                    w[:], tp[:], bias, None, op0=ALU.add
                                )
                                t2 = t2p.tile([128, 1024], f32, tag="t2")
                                nc.vector.scalar_tensor_tensor(
                                    t2[:], w[:], NEG_SLOPE, w[:],
                                    op0=ALU.mult, op1=ALU.max,
                                )
                                nc.scalar.activation(
                                    dst, t2[:], AF.Exp, bias=0.0, scale=1.0,
                                    accum_out=acc,
                                )

                    # ---- normalize: recip = 1/(sums+eps); whs = wh*recip ----
                    jsl = slice(jg * JPG, (jg + 1) * JPG)
                    nc.vector.tensor_reduce(
                        sums_red[:, jsl], sums_acc[:, jsl, :],
                        axis=mybir.AxisListType.X, op=ALU.add,
                    )
                    nc.vector.tensor_scalar(
                        sums_red[:, jsl], sums_red[:, jsl], EPS, None, op0=ALU.add
                    )
                    nc.vector.reciprocal(recip[:, jsl], sums_red[:, jsl])
                    whs = {}
                    for jl in range(JPG):
                        jt = jg * JPG + jl
                        ws16 = whsp.tile([128, F], f16, tag="whs")
                        nc.vector.tensor_scalar(
                            ws16[:], wh[jt][:], recip[:, jt : jt + 1], None,
                            op0=ALU.mult,
                        )
                        whs[jt] = ws16

                    # ---- partial output: out[i,:] += expsT^T @ whs ----
                    for it in range(NT):
                        po = outps.tile([128, F], f32)
                        for jl in range(JPG):
                            jt = jg * JPG + jl
                            nc.tensor.matmul(
                                po[:],
                                expsT[jt][:, it * 128 : (it + 1) * 128],
                                whs[jt][:],
                                start=(jl == 0), stop=(jl == JPG - 1),
                            )
                        if jg == 0:
                            nc.vector.tensor_copy(outsb[it][:], po[:])
                        else:
                            nc.vector.tensor_tensor(
                                outsb[it][:], po[:], outsb[it][:], op=ALU.add
                            )
                        if jg == NJG - 1:
                            nc.sync.dma_start(
                                out_d[it * 128 : (it + 1) * 128, :], outsb[it][:]
                            )

            if loop_n is None:
                body()
            else:
                with tc.For_i(0, loop_n, 1) as iv:
                    body(iv)

    nc.finalize()
    return nc


def _host_prep(A, x, W, a_w, a_b):
    """Per-core input maps from full inputs."""
    W64 = W.astype(np.float64)
    ha = W64 @ a_w[:F].astype(np.float64)
    hb = W64 @ a_w[F:].astype(np.float64)
    w16 = W.astype(np.float16)
    in_maps = []
    for b in range(B):
        xb = x[b]
        ssrc = (xb.astype(np.float64) @ ha).astype(np.float32)
        sdst = (xb.astype(np.float64) @ hb + float(a_b)).astype(np.float32)
        am2 = (A[b] - 1.0) * C2 + ssrc[:, None]
        sdstc = np.ascontiguousarray(sdst.reshape(NT, 128).T)
        xt16 = np.ascontiguousarray(xb.T).astype(np.float16)
        in_maps.append(
            {"am": am2.astype(np.float32), "xt": xt16, "w16": w16, "sdstc": sdstc}
        )
    return in_maps


_NC_CACHE = {}


def _get_nc(loop_n=None):
    key = loop_n
    if key not in _NC_CACHE:
        _NC_CACHE[key] = build(bacc.Bacc(), loop_n=loop_n)
    return _NC_CACHE[key]


def kernel(A, x, W, a_w, a_b):
    A = np.asarray(A, dtype=np.float32)
    x = np.asarray(x, dtype=np.float32)
    W = np.asarray(W, dtype=np.float32)
    a_w = np.asarray(a_w, dtype=np.float32)
    a_b = np.float32(a_b)
    nc = _get_nc()
    in_maps = _host_prep(A, x, W, a_w, a_b)
    res = run_bass_kernel_spmd(nc, in_maps, list(range(B)))
    return np.stack([res.results[b]["out"] for b in range(B)], axis=0)
